# revision 4
# baseline (speedup 1.0000x reference)
"""Trainium2 Bass kernel v3 for nn_DotAttention: softmax(Q @ V^T) @ V.

Sharding: 8 cores, each takes 2048 query rows of one batch + that batch's
full value tensor. Host pre-computes layouts (pure data movement):
  qts [128, 2048]: qT stacked twice (rows 0-63 = rows 64-127 = Q^T shard)
  vts [128, 2048]: top half = V^T even 128-tiles, bottom half = odd tiles
  vE  [4096, 65]:  V with a ones column appended

Per-core pipeline (chunk = 512 q, exp group = 3 v-tiles):
  mm1 (fp32r): scoresT[v,q] -> PSUM. Consecutive v-tiles use disjoint PE
    row-halves (tile_position (0,0)/(64,0)) so adjacent K=64 matmuls run
    concurrently in the array.
  exp (ScalarE): PSUM [128, 1536] -> SBUF f32r (ACT is the bottleneck
    engine: ~63us busy). No max-subtraction needed at these magnitudes.
  mm2 (fp32r): ctxT[65, 512] += V_ext^T @ expT; ones column accumulates
    the softmax denominator.
  epilogue: PE transpose ctxT -> [q, 65], DVE reciprocal+scale straight
    from PSUM, DMA out.
fp32r operands must be produced by a rounding instruction (walrus rule),
so each DMA'd piece gets one DVE tensor_copy fp32 -> f32r.
"""

import sys

sys.path.insert(0, "/opt/trn_rl_repo")

import numpy as np

import concourse.bass as bass  # noqa: F401
import concourse.mybir as mybir
import concourse.tile as tile
from concourse import bacc
from concourse.bass_utils import run_bass_kernel_spmd
from concourse.masks import make_identity

F32 = mybir.dt.float32
F32R = mybir.dt.float32r
EXP = mybir.ActivationFunctionType.Exp

B, TQ, TV, D = 4, 4096, 4096, 64
N_CORES = 8
QS = TQ * B // N_CORES  # 2048
CHUNK = 512
NCH = QS // CHUNK  # 4
NVT = TV // 128  # 32
VPIECE = 4  # v tiles per input piece
NVP = NVT // VPIECE  # 8 pieces

# exp groups per chunk: 10 groups of 3 v-tiles + 1 group of 2
GROUPS = [(s, min(3, NVT - s)) for s in range(0, NVT, 3)]
# emit V piece p one group before its first consumer (piece p holds
# tiles 4p..4p+3; group (g0,3) holds tiles g0..g0+2)
_PREFETCH = {}
for _p in range(1, NVP):
    _need = 3 * ((4 * _p - 2 + 2) // 3)  # first g0 with g0+2 >= 4p
    _PREFETCH.setdefault(max(_need - 3, 0), []).append(_p)

_cache = {}


def _build():
    nc = bacc.Bacc("TRN2", target_bir_lowering=False, debug=False)
    qt_d = nc.dram_tensor("qts", [128, QS], F32, kind="ExternalInput").ap()
    vt_d = nc.dram_tensor("vts", [128, TV // 2], F32, kind="ExternalInput").ap()
    ve_d = nc.dram_tensor("vE", [TV, D + 1], F32, kind="ExternalInput").ap()
    out = nc.dram_tensor("out", [QS, D], F32, kind="ExternalOutput").ap()

    with tile.TileContext(nc) as tc:
        with (
            tc.tile_pool(name="const", bufs=1) as const_pool,
            tc.tile_pool(name="stage", bufs=1) as stage_pool,
            tc.tile_pool(name="big", bufs=1) as big_pool,
            tc.tile_pool(name="sb", bufs=1) as sb_pool,
            tc.tile_pool(name="ps", bufs=1, space="PSUM") as ps_pool,
            tc.tile_pool(name="pst", bufs=1, space="PSUM") as pst_pool,
        ):
            ident = const_pool.tile([128, 128], F32)
            make_identity(nc, ident)

            # PE observes the identity's (gpsimd) semaphore once so later
            # transposes carry fewer waits
            warm = pst_pool.tile([128, 128], F32, tag="tr", bufs=1)
            nc.tensor.transpose(warm, ident, ident)

            vts = big_pool.tile([128, TV // 2], F32R, tag="vts")
            qts = big_pool.tile([128, QS], F32R, tag="qts")
            ve_list = []

            def v_piece(p):
                """DMA vts piece (pairs 2p..2p+1) + v_ext piece, round."""
                c0, c1 = 2 * p * 128, (2 * p + 2) * 128
                vs = stage_pool.tile(
                    [128, 256], F32, tag="vstage", bufs=3, name=f"vs{p}"
                )
                nc.sync.dma_start(out=vs, in_=vt_d[:, c0:c1])
                nc.vector.tensor_copy(vts[:, c0:c1], vs)
                t0 = p * VPIECE * 128
                es = stage_pool.tile(
                    [128, VPIECE, D + 1], F32, tag="estage", bufs=3, name=f"es{p}"
                )
                nc.scalar.dma_start(
                    out=es,
                    in_=ve_d[t0 : t0 + VPIECE * 128, :].rearrange(
                        "(t p) e -> p t e", p=128
                    ),
                )
                ve = big_pool.tile(
                    [128, VPIECE, D + 1], F32R, tag=f"ve{p}", bufs=1, name=f"ve{p}"
                )
                ve_list.append(ve)
                nc.vector.tensor_copy(ve, es)

            def q_piece(ch):
                c0, c1 = ch * CHUNK, (ch + 1) * CHUNK
                qs_ = stage_pool.tile(
                    [128, CHUNK], F32, tag="qstage", bufs=2, name=f"qs{ch}"
                )
                nc.sync.dma_start(out=qs_, in_=qt_d[:, c0:c1])
                nc.vector.tensor_copy(qts[:, c0:c1], qs_)

            q_piece(0)
            v_piece(0)

            for ch in range(NCH):
                ctx_ps = ps_pool.tile(
                    [D + 1, CHUNK], F32, tag="ctx", bufs=1, name=f"ctx{ch}"
                )
                for gi, (g0, gn) in enumerate(GROUPS):
                    if ch == 0:
                        for p in _PREFETCH.get(g0, []):
                            v_piece(p)
                    if gi == 5 and ch + 1 < NCH:
                        q_piece(ch + 1)
                    sc = ps_pool.tile(
                        [128, gn, CHUNK], F32, tag="scores", bufs=2,
                        padded_shape=[128, 3, CHUNK], name=f"sc{ch}_{g0}",
                    )
                    for t in range(gn):
                        i = g0 + t
                        h = i % 2
                        pair = i // 2
                        nc.tensor.matmul(
                            sc[:, t, :],
                            vts[h * 64 : (h + 1) * 64, pair * 128 : (pair + 1) * 128],
                            qts[h * 64 : (h + 1) * 64, ch * CHUNK : (ch + 1) * CHUNK],
                            start=True,
                            stop=True,
                            tile_position=(h * 64, 0),
                        )
                    et = sb_pool.tile(
                        [128, gn, CHUNK], F32R, tag="expt", bufs=3,
                        padded_shape=[128, 3, CHUNK], name=f"et{ch}_{g0}",
                    )
                    nc.scalar.activation(et, sc, EXP)
                    for t in range(gn):
                        i = g0 + t
                        nc.tensor.matmul(
                            ctx_ps,
                            ve_list[i // VPIECE][:, i % VPIECE, :],
                            et[:, t, :],
                            start=(i == 0),
                            stop=(i == NVT - 1),
                        )
                # epilogue: transpose ctxT back, normalize from PSUM, store
                ctxt = sb_pool.tile(
                    [D + 1, CHUNK], F32, tag="ctxt", bufs=2, name=f"ctxt{ch}"
                )
                nc.vector.tensor_copy(ctxt, ctx_ps)
                for j in range(CHUNK // 128):
                    tr = pst_pool.tile(
                        [128, D + 1], F32, tag="tr", bufs=1, name=f"tr{ch}_{j}"
                    )
                    nc.tensor.transpose(
                        tr, ctxt[:, j * 128 : (j + 1) * 128], ident[: D + 1, : D + 1]
                    )
                    rec = sb_pool.tile([128, 1], F32, tag="rec", bufs=2)
                    nc.vector.reciprocal(rec, tr[:, D : D + 1])
                    o_n = sb_pool.tile([128, D], F32, tag="on", bufs=2)
                    nc.vector.tensor_scalar_mul(o_n, tr[:, :D], rec)
                    nc.sync.dma_start(
                        out=out[ch * CHUNK + j * 128 : ch * CHUNK + (j + 1) * 128, :],
                        in_=o_n,
                    )

    nc.compile()
    return nc


def _get_nc():
    if "nc" not in _cache:
        _cache["nc"] = _build()
    return _cache["nc"]


def kernel(query: np.ndarray, value: np.ndarray, **run_kwargs) -> np.ndarray:
    query = np.asarray(query, dtype=np.float32)
    value = np.asarray(value, dtype=np.float32)
    nc = _get_nc()
    shards_per_b = N_CORES // B
    in_maps = []
    ones = np.ones((TV, 1), dtype=np.float32)
    for c in range(N_CORES):
        b, s = divmod(c, shards_per_b)
        qT = query[b, s * QS : (s + 1) * QS].T  # [64, QS]
        vT3 = value[b].T.reshape(D, NVT, 128)  # [64, 32, 128]
        vts = np.concatenate(
            [
                vT3[:, 0::2, :].reshape(D, TV // 2),
                vT3[:, 1::2, :].reshape(D, TV // 2),
            ],
            axis=0,
        )
        in_maps.append(
            {
                "qts": np.ascontiguousarray(np.concatenate([qT, qT], axis=0)),
                "vts": np.ascontiguousarray(vts),
                "vE": np.ascontiguousarray(np.concatenate([value[b], ones], axis=1)),
            }
        )
    res = run_bass_kernel_spmd(nc, in_maps, core_ids=list(range(N_CORES)), **run_kwargs)
    _cache["last_results"] = res
    out = np.empty((B, TQ, D), dtype=np.float32)
    for c in range(N_CORES):
        b, s = divmod(c, shards_per_b)
        out[b, s * QS : (s + 1) * QS] = res.results[c]["out"]
    return out


# revision 11
# speedup vs baseline: 1.0425x; 1.0425x over previous
"""Trainium2 Bass kernel v3 for nn_DotAttention: softmax(Q @ V^T) @ V.

Sharding: 8 cores, each takes 2048 query rows of one batch + that batch's
full value tensor. Host pre-computes layouts (pure data movement):
  qts [128, 2048]: qT stacked twice (rows 0-63 = rows 64-127 = Q^T shard)
  vts [128, 2048]: top half = V^T even 128-tiles, bottom half = odd tiles
  vE  [4096, 65]:  V with a ones column appended

Per-core pipeline (chunk = 512 q, exp group = 3 v-tiles):
  mm1 (fp32r): scoresT[v,q] -> PSUM. Consecutive v-tiles use disjoint PE
    row-halves (tile_position (0,0)/(64,0)) so adjacent K=64 matmuls run
    concurrently in the array.
  exp (ScalarE): PSUM [128, 1536] -> SBUF f32r (ACT is the bottleneck
    engine: ~63us busy). No max-subtraction needed at these magnitudes.
  mm2 (fp32r): ctxT[65, 512] += V_ext^T @ expT; ones column accumulates
    the softmax denominator.
  epilogue: PE transpose ctxT -> [q, 65], DVE reciprocal+scale straight
    from PSUM, DMA out.
fp32r operands must be produced by a rounding instruction (walrus rule),
so each DMA'd piece gets one DVE tensor_copy fp32 -> f32r.
"""

import sys

sys.path.insert(0, "/opt/trn_rl_repo")

import numpy as np

import concourse.bass as bass  # noqa: F401
import concourse.mybir as mybir
import concourse.tile as tile
from concourse import bacc
from concourse.bass_utils import run_bass_kernel_spmd
from concourse.masks import make_identity

F32 = mybir.dt.float32
F32R = mybir.dt.float32r
EXP = mybir.ActivationFunctionType.Exp

B, TQ, TV, D = 4, 4096, 4096, 64
N_CORES = 8
QS = TQ * B // N_CORES  # 2048
CHUNK = 512
NCH = QS // CHUNK  # 4
NVT = TV // 128  # 32
VPIECE = 4  # v tiles per input piece
NVP = NVT // VPIECE  # 8 pieces

# exp groups per chunk: 10 groups of 3 v-tiles + 1 group of 2; chunk 0
# starts with a single-tile group so the first exp fires ~4us earlier
GROUPS = [(s, min(3, NVT - s)) for s in range(0, NVT, 3)]
GROUPS0 = [(0, 1)] + [(s, 3) for s in range(1, NVT - 1, 3)] + [(NVT - 1, 1)]

_cache = {}


def _build():
    nc = bacc.Bacc("TRN2", target_bir_lowering=False, debug=False)
    qt_d = nc.dram_tensor("qts", [128, QS], F32, kind="ExternalInput").ap()
    vt_d = nc.dram_tensor("vts", [128, TV // 2], F32, kind="ExternalInput").ap()
    ve_d = nc.dram_tensor("vE", [128, NVT * (D + 1)], F32, kind="ExternalInput").ap()
    out = nc.dram_tensor("out", [QS, D], F32, kind="ExternalOutput").ap()

    with tile.TileContext(nc) as tc:
        with (
            tc.tile_pool(name="const", bufs=1) as const_pool,
            tc.tile_pool(name="stage", bufs=1) as stage_pool,
            tc.tile_pool(name="big", bufs=1) as big_pool,
            tc.tile_pool(name="sb", bufs=1) as sb_pool,
            tc.tile_pool(name="ps", bufs=1, space="PSUM") as ps_pool,
            tc.tile_pool(name="pst", bufs=1, space="PSUM") as pst_pool,
        ):
            ident = const_pool.tile([128, 128], F32)
            make_identity(nc, ident)

            # PE observes the identity's (gpsimd) semaphore once so later
            # transposes carry fewer waits
            warm = pst_pool.tile([128, 128], F32, tag="tr", bufs=1)
            nc.tensor.transpose(warm, ident, ident)

            vts = big_pool.tile([128, TV // 2], F32R, tag="vts")
            qts = big_pool.tile([128, QS], F32R, tag="qts")
            ve_list = []

            def v_piece(p):
                """DMA vts piece (pairs 2p..2p+1) + v_ext piece, round."""
                c0, c1 = 2 * p * 128, (2 * p + 2) * 128
                vs = stage_pool.tile(
                    [128, 256], F32, tag="vstage", bufs=4, name=f"vs{p}"
                )
                nc.sync.dma_start(out=vs, in_=vt_d[:, c0:c1])
                nc.vector.tensor_copy(vts[:, c0:c1], vs)
                e0 = p * VPIECE * (D + 1)
                es = stage_pool.tile(
                    [128, VPIECE, D + 1], F32, tag="estage", bufs=4, name=f"es{p}"
                )
                nc.gpsimd.dma_start(
                    out=es, in_=ve_d[:, e0 : e0 + VPIECE * (D + 1)]
                )
                ve = big_pool.tile(
                    [128, VPIECE, D + 1], F32R, tag=f"ve{p}", bufs=1, name=f"ve{p}"
                )
                ve_list.append(ve)
                nc.vector.tensor_copy(ve, es)

            def q_piece(ch):
                c0, c1 = ch * CHUNK, (ch + 1) * CHUNK
                qs_ = stage_pool.tile(
                    [128, CHUNK], F32, tag="qstage", bufs=2, name=f"qs{ch}"
                )
                nc.sync.dma_start(out=qs_, in_=qt_d[:, c0:c1])
                nc.vector.tensor_copy(qts[:, c0:c1], qs_)

            # startup order matters: DVE executes copies in program
            # order, so vts0 and qts0 must precede everything else
            vs0 = stage_pool.tile([128, 256], F32, tag="vstage", bufs=4)
            nc.sync.dma_start(out=vs0, in_=vt_d[:, :256])
            nc.vector.tensor_copy(vts[:, :256], vs0)
            q_piece(0)
            es0 = stage_pool.tile(
                [128, VPIECE, D + 1], F32, tag="estage", bufs=4
            )
            nc.gpsimd.dma_start(out=es0, in_=ve_d[:, : VPIECE * (D + 1)])
            ve0 = big_pool.tile([128, VPIECE, D + 1], F32R, tag="ve0", bufs=1)
            ve_list.append(ve0)
            nc.vector.tensor_copy(ve0, es0)
            v_piece(1)
            v_piece(2)

            def epilogue(ch, ctx_ps):
                # transpose ctxT back, normalize straight from PSUM, store.
                # Last chunk: route trs through the (now idle) scores slots
                # so the final chain pipelines instead of serializing.
                last = ch == NCH - 1
                ctxt = sb_pool.tile(
                    [D + 1, CHUNK], F32, tag="ctxt", bufs=2, name=f"ctxt{ch}"
                )
                nc.vector.tensor_copy(ctxt, ctx_ps)
                for j in range(CHUNK // 128):
                    if last:
                        tr = ps_pool.tile(
                            [128, D + 1], F32, tag="scores", bufs=2,
                            padded_shape=[128, 3 * CHUNK], name=f"tr{ch}_{j}",
                        )
                    else:
                        tr = pst_pool.tile(
                            [128, D + 1], F32, tag="tr", bufs=1, name=f"tr{ch}_{j}"
                        )
                    nc.tensor.transpose(
                        tr, ctxt[:, j * 128 : (j + 1) * 128], ident[: D + 1, : D + 1]
                    )
                    rec = sb_pool.tile([128, 1], F32, tag="rec", bufs=4)
                    nc.vector.reciprocal(rec, tr[:, D : D + 1])
                    o_n = sb_pool.tile([128, D], F32, tag="on", bufs=4)
                    nc.vector.tensor_scalar_mul(o_n, tr[:, :D], rec)
                    nc.sync.dma_start(
                        out=out[ch * CHUNK + j * 128 : ch * CHUNK + (j + 1) * 128, :],
                        in_=o_n,
                    )

            def emit_mm2(ctx_ps, g0, gn, et):
                for t in range(gn):
                    i = g0 + t
                    nc.tensor.matmul(
                        ctx_ps,
                        ve_list[i // VPIECE][:, i % VPIECE, :],
                        et[:, t, :],
                        start=(i == 0),
                        stop=(i == NVT - 1),
                    )

            pending = None
            pend_mm2 = None  # mm2 runs one group behind its exp (keeps
            pieces_emitted = 3  # mm1(g+1) off exp(g)'s critical path)
            ctx_by_ch = {}
            for ch in range(NCH):
                ctx_by_ch[ch] = ps_pool.tile(
                    [D + 1, CHUNK], F32, tag="ctx", bufs=1, name=f"ctx{ch}"
                )
                groups = GROUPS0 if ch == 0 else GROUPS
                for gi, (g0, gn) in enumerate(groups):
                    if ch == 0:
                        want = (min(g0 + gn + 6, NVT) - 1) // VPIECE
                        while pieces_emitted <= want:
                            v_piece(pieces_emitted)
                            pieces_emitted += 1
                    if gi == 2 and pending is not None:
                        # previous chunk's epilogue, off the critical path
                        epilogue(*pending)
                        pending = None
                    if gi == 5 and ch + 1 < NCH:
                        q_piece(ch + 1)
                    sc = ps_pool.tile(
                        [128, gn, CHUNK], F32, tag="scores", bufs=2,
                        padded_shape=[128, 3, CHUNK], name=f"sc{ch}_{g0}",
                    )
                    for t in range(gn):
                        i = g0 + t
                        h = i % 2
                        pair = i // 2
                        nc.tensor.matmul(
                            sc[:, t, :],
                            vts[h * 64 : (h + 1) * 64, pair * 128 : (pair + 1) * 128],
                            qts[h * 64 : (h + 1) * 64, ch * CHUNK : (ch + 1) * CHUNK],
                            start=True,
                            stop=True,
                            tile_position=(h * 64, 0),
                        )
                    et = sb_pool.tile(
                        [128, gn, CHUNK], F32R, tag="expt", bufs=3,
                        padded_shape=[128, 3, CHUNK], name=f"et{ch}_{g0}",
                    )
                    nc.scalar.activation(et, sc, EXP)
                    if pend_mm2 is not None:
                        emit_mm2(*pend_mm2)
                    pend_mm2 = (ctx_by_ch[ch], g0, gn, et)
                if ch == NCH - 1:
                    emit_mm2(*pend_mm2)
                    pend_mm2 = None
                    epilogue(ch, ctx_by_ch[ch])
                else:
                    pending = (ch, ctx_by_ch[ch])

    nc.compile()
    return nc


def _get_nc():
    if "nc" not in _cache:
        _cache["nc"] = _build()
    return _cache["nc"]


def kernel(query: np.ndarray, value: np.ndarray, **run_kwargs) -> np.ndarray:
    query = np.asarray(query, dtype=np.float32)
    value = np.asarray(value, dtype=np.float32)
    nc = _get_nc()
    shards_per_b = N_CORES // B
    in_maps = []
    ones = np.ones((TV, 1), dtype=np.float32)
    for c in range(N_CORES):
        b, s = divmod(c, shards_per_b)
        qT = query[b, s * QS : (s + 1) * QS].T  # [64, QS]
        vT3 = value[b].T.reshape(D, NVT, 128)  # [64, 32, 128]
        vts = np.concatenate(
            [
                vT3[:, 0::2, :].reshape(D, TV // 2),
                vT3[:, 1::2, :].reshape(D, TV // 2),
            ],
            axis=0,
        )
        in_maps.append(
            {
                "qts": np.ascontiguousarray(np.concatenate([qT, qT], axis=0)),
                "vts": np.ascontiguousarray(vts),
                "vE": np.ascontiguousarray(
                    np.concatenate([value[b], ones], axis=1)
                    .reshape(NVT, 128, D + 1)
                    .transpose(1, 0, 2)
                    .reshape(128, NVT * (D + 1))
                ),
            }
        )
    res = run_bass_kernel_spmd(nc, in_maps, core_ids=list(range(N_CORES)), **run_kwargs)
    _cache["last_results"] = res
    out = np.empty((B, TQ, D), dtype=np.float32)
    for c in range(N_CORES):
        b, s = divmod(c, shards_per_b)
        out[b, s * QS : (s + 1) * QS] = res.results[c]["out"]
    return out
